# revision 63
# baseline (speedup 1.0000x reference)
"""Distributed Trainium2 Bass kernel for masked multi-head attention.

Problem: out = (softmax(scale * x Wq^T (x Wk^T)^T + mask * -1e5) (x Wv^T)) Wp^T + bp
  x [4, 2048, 768], mask [4, 2048, 2048], H=12 heads, D=64.

Sharding (8 cores): core = (batch b, head-group hg) with b = core//2,
hg = core%2 (6 heads each).  Column-parallel Wq/Wkv, row-parallel Wp;
each core produces a partial [2048, 768] output; the host sums the two
head-group partials per batch and adds the bias, then stacks batches.

Device pipeline per core (PE busy ~230us is the HW floor for this
decomposition; the schedule aims to make the span equal PE busy):
  phase 1: ALL Q/K/V projections up-front (PE is 100% busy on dense
    GEMMs; the PSUM evictions land on ACT/DVE while they are otherwise
    idle, instead of disturbing the attention steady state).
    Chunking the x DMA by column blocks measured slower -- keep whole-
    tile transfers.  Starting attention early (K+V+first Q columns
    only, remaining Q projected mid-attention through ring psums)
    measured +60us: with consumers ~89% busy and a 2-slot ring, ANY
    mid-attention ring insertion or consumer-work injection stalls the
    pipeline (same failure as Pool muls and mid-chunk pps).
  phase 2: per (head, k-tile pair): S^T tile = K^T slice x Q^T into a
    2-slot 2-bank PSUM ring ([128, 2, 512] per slot).  The steady-state
    clock is the exp issue rate (the ring recycles on exp and P@V gates
    on the mask-mul behind it): measured ACT exp 986ns vs DVE
    Schraudolph (tensor_scalar mult SCH_A add SCH_B, int16 out, 2x
    mode) 526ns per pair, so every 4th pair's exp goes to DVE, cutting
    the ACT stream to ~740ns/pair; with projections up-front DVE has
    room for its exp share plus ALL mask multiplies (~585ns, 2x mode).
    O += P^T slices @ [V | 1] on PE (denominator = 65th col).
    Measured dead ends: activation with scale!=1 is ~1.3x slower
    (keep ACT exp unscaled); Pool (gpsimd) muls are 2.1us nominal but
    3-6us under SBUF contention and sit on the PV gate -- keep Pool
    out of the steady state; scalar_tensor_tensor add,min fusion runs
    at 0.78x (slower than the two-op split); [128,512]-grain consumer
    ops pay ~2x fixed overhead; QCHUNK=256 adds +34us of PE overhead.
    ACT_TABLE_LOAD fires once: Exp and Copy share a table set, so ACT
    may do epilogue copies.
  epilogue per q-chunk: normalize at the NEXT chunk's top (so the
    O-accumulator WAR resolves early in the DVE queue), transpose +
    out-project at the next chunk's first-head end.  The out-proj pps
    share the o_pool with the O accumulators, so each chunk's
    projection actually executes ~a chunk late -- acceptable overflow
    work for the elementwise-bound steady state; every attempt to
    borrow ring slots for it instead (mid-chunk, inline at chunk end,
    or for the final chunk only) measured 25-35us/boundary stalls or
    run-to-run bimodality (+52us half the time).  Next chunk's mask is
    prefetched at h==1 so the 2MB of mask DMA queues behind the
    epilogue transposes on the sync engine, not ahead of them.
PSUM: 2x2-bank ring + 4 O-accumulator banks.  fp8/DoubleRow matmuls
are NOT faster than bf16 on TRN2, fp8 attention weights fail accuracy,
64-partition QK operands measured slower than padded-128 ones, and DMA
and GPSIMD physically cannot touch PSUM (no additive pre-exp mask).
Steady state is ACT+DVE throughput-bound (~365us of elementwise work
over two engines vs ~230us PE), so the span floor is ~290us; measured
293.8-294.3us across repeated runs with PV_LAG=4 and the mask-mul
deferred TWO pairs behind its exp (baseline 302-304us).  The machine
itself is bimodal: some runs are ~1.2x slower (+50us) -- verify any
change with 2-3 runs.
"""

import os
from contextlib import ExitStack

import ml_dtypes
import numpy as np

import sys
import types

try:  # defensive: concourse's trace path imports this; absent on some images
    import antenv.axon_hooks  # noqa: F401
except ImportError:
    try:
        import antenv
        _m = types.ModuleType('antenv.axon_hooks')
        _m._hook = None
        _m.set_axon_ntff_profile_hook = lambda h: setattr(_m, '_hook', h)
        _m.get_axon_ntff_profile_hook = lambda: _m._hook
        sys.modules['antenv.axon_hooks'] = _m
        antenv.axon_hooks = _m
    except ImportError:
        pass

import concourse.bass as bass
import concourse.tile as tile
from concourse import bacc, mybir
from concourse.bass_utils import run_bass_kernel_spmd

B, N, C, H, D = 4, 2048, 768, 12, 64
SCALE = D ** -0.5
NCORES = 8
HGROUPS = 2
HL = H // HGROUPS          # 6 heads per group
CH = HL * D                # 384 channels per group
P = 128
NKT = N // P               # 16 k tiles
QCHUNK = 512
NQC = N // QCHUNK          # 4 q chunks
QSUBS = QCHUNK // P        # 4
CIN_T = C // P             # 6 input-channel tiles
CH_T = CH // P             # 3 group-channel tiles
E = D + 1                  # head slot width in O psum (64 V cols + 1 ones col)

F32 = mybir.dt.float32
BF16 = mybir.dt.bfloat16
I16 = mybir.dt.int16

# Schraudolph exp emitting bf16 bits (input already scaled by SCALE):
#   bf16bits(exp(x)) ~= round((2^23/ln2 * x + 127*2^23 - 486411) / 2^16)
SCH_A = (2.0 ** 23 / np.log(2.0)) / 2.0 ** 16
SCH_B = (127 * 2 ** 23 - 486411) / 2.0 ** 16
PV_LAG = 4                 # software-pipeline depth for the P@V stage (pairs)
RING_BUFS = 2
EXP_DVE_PERIOD = 5         # every 5th pair's exp on DVE: an ODD period
                           # de-aliases the DVE exps from the 2-slot
                           # ring parity (even periods pin every DVE exp
                           # to slot 1, leaving slot 0's recycle always
                           # gated by the slower ACT exp)


def build_kernel():
    nc = bacc.Bacc("TRN2", target_bir_lowering=False, debug=False,
                   num_devices=NCORES)

    xT = nc.dram_tensor("xT", [C, N], BF16, kind="ExternalInput").ap()
    wqt = nc.dram_tensor("wqt", [C, CH], BF16, kind="ExternalInput").ap()
    wkt = nc.dram_tensor("wkt", [C, CH], BF16, kind="ExternalInput").ap()
    wvt = nc.dram_tensor("wvt", [C, CH], BF16, kind="ExternalInput").ap()
    wpt = nc.dram_tensor("wpt", [CH, C], BF16, kind="ExternalInput").ap()
    negmt = nc.dram_tensor("negmt", [N, N], BF16, kind="ExternalInput").ap()
    out = nc.dram_tensor("out", [N, C], F32, kind="ExternalOutput").ap()

    with tile.TileContext(nc) as tc, ExitStack() as ctx:
        persist = ctx.enter_context(tc.tile_pool(name="persist", bufs=1))
        # PSUM pools: "ring" slots are 2 banks each, "ot" slots 1 bank
        # each -> 2*2 + 4*1 = 8 banks total.
        ring_pool = ctx.enter_context(
            tc.tile_pool(name="ring", bufs=RING_BUFS, space="PSUM"))
        o_pool = ctx.enter_context(
            tc.tile_pool(name="opsum", bufs=4, space="PSUM"))

        qt_all = persist.tile([P, HL, N], BF16, tag="qta", name="qta")
        kt_all = persist.tile([P, HL, N], BF16, tag="kta", name="kta")
        qt_sb = [qt_all[:, i, :] for i in range(HL)]
        kt_sb = [kt_all[:, i, :] for i in range(HL)]
        vp_sb = [persist.tile([P, HL, E], BF16, tag=f"vp{j}", name=f"vp{j}")
                 for j in range(NKT)]
        wp_sb = [persist.tile([P, C], BF16, tag=f"wp{t}", name=f"wp{t}")
                 for t in range(CH_T)]

        # Zero the pad rows (64..127) of QT/KT on Pool (idle in phase 1).
        nc.gpsimd.memset(kt_all[D:P, :, :], 0.0)
        nc.gpsimd.memset(qt_all[D:P, :, :], 0.0)

        # ---- phase 1: projections (all up-front) ----
        ph1 = ctx.enter_context(tc.tile_pool(name="ph1", bufs=1))
        xt_sb = [ph1.tile([P, N], BF16, tag=f"xt{i}", name=f"xt{i}")
                 for i in range(CIN_T)]
        wq_sb = [ph1.tile([P, CH], BF16, tag=f"wq{i}", name=f"wq{i}")
                 for i in range(CIN_T)]
        wk_sb = [ph1.tile([P, CH], BF16, tag=f"wk{i}", name=f"wk{i}")
                 for i in range(CIN_T)]
        wv_sb = [ph1.tile([P, CH], BF16, tag=f"wv{i}", name=f"wv{i}")
                 for i in range(CIN_T)]
        for i in range(CIN_T):
            sl = slice(i * P, (i + 1) * P)
            nc.sync.dma_start(out=xt_sb[i], in_=xT[sl, :])
            nc.sync.dma_start(out=wq_sb[i], in_=wqt[sl, :])
        for i in range(CIN_T):
            sl = slice(i * P, (i + 1) * P)
            nc.sync.dma_start(out=wk_sb[i], in_=wkt[sl, :])
            nc.sync.dma_start(out=wv_sb[i], in_=wvt[sl, :])
        for t in range(CH_T):
            nc.sync.dma_start(out=wp_sb[t], in_=wpt[t * P:(t + 1) * P, :])

        # V: [N, CH] = x @ Wv^T into the ones-augmented bf16 layout
        # vp[j] = [P, HL, 65] with vp[..., 64] == 1.0.
        for j in range(NKT):
            nc.gpsimd.memset(vp_sb[j], 1.0)
        try:  # preload the gpsimd library so no mid-stream reload stalls Pool
            from concourse import library_config
            nc.gpsimd.load_library(library_config.standard)
        except Exception:
            pass

        def emit_qtkt_nck(m, which, nck, pool, tg):
            dst, w_sb = (qt_sb, wq_sb) if which == 0 else (kt_sb, wk_sb)
            ps = pool.tile([P, 512], F32, tag=tg,
                           name=f"p1ps{m}_{which}_{nck}")
            for ci in range(CIN_T):
                nc.tensor.matmul(
                    ps,
                    w_sb[ci][:, m * P:(m + 1) * P],
                    xt_sb[ci][:, nck * 512:(nck + 1) * 512],
                    start=(ci == 0), stop=(ci == CIN_T - 1))
            for sub in range(2):
                dtile = dst[2 * m + sub]
                dslice = dtile[0:D, nck * 512:(nck + 1) * 512]
                pslice = ps[sub * D:(sub + 1) * D, :]
                if which == 0:
                    nc.scalar.copy(dslice, pslice)
                else:
                    nc.vector.tensor_copy(dslice, pslice)

        alt = 0

        def p1pool():
            nonlocal alt
            alt += 1
            return ((ring_pool, "ring") if alt % 2 == 0 else (o_pool, "ot"))

        for m in range(CH_T):
            for nck in range(N // 512):
                emit_qtkt_nck(m, 0, nck, *p1pool())
        for m in range(CH_T):
            for nck in range(N // 512):
                emit_qtkt_nck(m, 1, nck, *p1pool())
        for j in range(NKT):
            pool, tg = p1pool()
            ps = pool.tile([P, CH], F32, tag=tg, name=f"vps{j}")
            for ci in range(CIN_T):
                nc.tensor.matmul(
                    ps,
                    xt_sb[ci][:, j * P:(j + 1) * P],
                    wv_sb[ci],
                    start=(ci == 0), stop=(ci == CIN_T - 1))
            nc.vector.tensor_copy(
                vp_sb[j][:, :, 0:D],
                ps.rearrange("p (h d) -> p h d", h=HL))

        # ---- phase 2: attention ----
        mpool = ctx.enter_context(tc.tile_pool(name="mask", bufs=2))
        p_pool = ctx.enter_context(tc.tile_pool(name="pexp", bufs=8))
        pm_pool = ctx.enter_context(tc.tile_pool(name="pmask", bufs=8))
        epi = ctx.enter_context(tc.tile_pool(name="epi", bufs=8))
        ot_pool = ctx.enter_context(tc.tile_pool(name="otsb", bufs=2))
        outsb_pool = ctx.enter_context(tc.tile_pool(name="outsb", bufs=4))

        def make_epilogue(qc, q0, otiles):
            osbs = {}

            def head_s(s):
                # normalize O and release the otile PSUM bank (DVE only)
                otv = otiles[s].rearrange("p (h e) -> p h e", h=HL)
                zrec = epi.tile([P, HL], F32, tag="zr", name=f"zr{qc}_{s}")
                nc.vector.reciprocal(zrec, otv[:, :, D])
                osb = epi.tile([P, HL, D], BF16, tag="osb",
                               name=f"osb{qc}_{s}")
                zb = bass.AP(
                    tensor=zrec.tensor, offset=zrec.offset,
                    ap=[*zrec.ap, [0, D]])
                nc.vector.tensor_mul(osb, otv[:, :, 0:D], zb)
                osbs[s] = osb

            def trans_s(s, otsb):
                osf = osbs[s].rearrange("p h d -> p (h d)")
                for ct in range(CH_T):
                    nc.sync.dma_start_transpose(
                        otsb[:, ct, s * P:(s + 1) * P],
                        osf[:, ct * P:(ct + 1) * P])

            def proj_s(s, otsb, pool, tg):
                # Mid-stream pps MUST come from o_pool: ring borrowing
                # there stalls subsequent QKs behind ob copies (26us per
                # boundary measured); inline-at-chunk-end epilogues cost
                # 24-36us per boundary the same way.  The o_pool
                # rotation runs each chunk's out-projection late, but
                # that overflow work is absorbed by the elementwise-
                # bound steady state (pre-allocating pps ahead of the
                # next chunk's otiles measured neutral -- conservation).
                for cf, (c0, c1) in enumerate(((0, CH), (CH, C))):
                    pps = pool.tile([P, CH], F32, tag=tg,
                                    name=f"pps{qc}_{s}_{cf}")
                    for ct in range(CH_T):
                        nc.tensor.matmul(
                            pps,
                            otsb[:, ct, s * P:(s + 1) * P],
                            wp_sb[ct][:, c0:c1],
                            start=(ct == 0), stop=(ct == CH_T - 1))
                    if cf == 0:
                        ob = outsb_pool.tile([P, C], F32, tag="ob",
                                             name=f"ob{qc}_{s}")
                        nc.vector.tensor_copy(ob[:, c0:c1], pps)
                    else:
                        # split DVE/ACT: both-on-ACT measured bimodal
                        # (295 vs 348us run-to-run)
                        nc.scalar.copy(ob[:, c0:c1], pps)
                nc.sync.dma_start(
                    out=out[q0 + s * P:q0 + (s + 1) * P, :], in_=ob)

            def epi_head():
                # chunk top: normalize, freeing the otile banks early
                for s in range(QSUBS):
                    head_s(s)

            def epi_tail():
                otsb = ot_pool.tile([P, CH_T, QCHUNK], BF16, tag="otsb",
                                    name=f"otsb{qc}")
                for s in range(QSUBS):
                    trans_s(s, otsb)
                for s in range(QSUBS):
                    proj_s(s, otsb, o_pool, "ot")

            def epi_fused():
                # final chunk: pipeline per subtile so the tail chain is
                # normalize(s0) -> transpose(s0) -> outproj(s0) while
                # s1..s3 normalize behind it, instead of stage barriers.
                otsb = ot_pool.tile([P, CH_T, QCHUNK], BF16, tag="otsb",
                                    name=f"otsb{qc}")
                for s in range(QSUBS):
                    head_s(s)
                    trans_s(s, otsb)
                    # o_pool here too: every ring-borrowing epilogue
                    # variant measured bimodal (~+52us half the time)
                    proj_s(s, otsb, o_pool, "ot")

            return epi_head, epi_tail, epi_fused

        pending_epi = None
        pending_pv = []
        pending_mul = []

        def flush_mul(keep=0):
            while len(pending_mul) > keep:
                pending_mul.pop(0)()

        def flush_pv(keep=0):
            while len(pending_pv) > keep:
                pending_pv.pop(0)()

        def dma_mask(qc):
            mk = mpool.tile([P, NKT, QCHUNK], BF16, tag="mk",
                            name=f"mk{qc}")
            for j in range(NKT):
                nc.sync.dma_start(
                    out=mk[:, j, :],
                    in_=negmt[j * P:(j + 1) * P,
                              qc * QCHUNK:(qc + 1) * QCHUNK])
            return mk

        ectr = 0
        mk_next = dma_mask(0)
        for qc in range(NQC):
            q0 = qc * QCHUNK
            if pending_epi is not None:
                pending_epi[0]()
            mk = mk_next

            otiles = [o_pool.tile([P, HL * E], F32, tag="ot",
                                  name=f"otile{qc}_{s_}")
                      for s_ in range(QSUBS)]

            for h in range(HL):
                kth = kt_sb[h]
                qth = qt_sb[h]
                for ktp in range(NKT // 2):
                    ring = ring_pool.tile([P, 2, QCHUNK], F32, tag="ring")
                    for u in range(2):
                        kti = 2 * ktp + u
                        nc.tensor.matmul(
                            ring[:, u, :],
                            kth[:, kti * P:(kti + 1) * P],
                            qth[:, q0:q0 + QCHUNK],
                            start=True, stop=True)
                    mslice = mk[:, 2 * ktp:2 * ktp + 2, :]
                    pexp = p_pool.tile([P, 2, QCHUNK], BF16, tag="pe")
                    pm = pm_pool.tile([P, 2, QCHUNK], BF16, tag="pm")
                    if ectr % EXP_DVE_PERIOD == EXP_DVE_PERIOD - 1:
                        # NOTE: moving this pair's mul to Pool regressed
                        # 297->377us (as did every Pool-in-the-mul-path
                        # variant); gpsimd stalls under SBUF contention.
                        nc.vector.tensor_scalar(
                            out=pexp.bitcast(I16), in0=ring,
                            scalar1=float(SCH_A), scalar2=float(SCH_B),
                            op0=mybir.AluOpType.mult,
                            op1=mybir.AluOpType.add)
                    else:
                        nc.scalar.activation(
                            pexp, ring, mybir.ActivationFunctionType.Exp)
                    ectr += 1
                    # defer the mul by one pair: ring-freeing exps then
                    # jump ahead of muls in DVE's in-order queue
                    pending_mul.append(
                        lambda pm=pm, pexp=pexp, mslice=mslice:
                        nc.vector.tensor_mul(pm, pexp, mslice))
                    flush_mul(keep=3)

                    def pv_fn(pm=pm, h=h, ktp=ktp, otiles=otiles):
                        for u in range(2):
                            kti = 2 * ktp + u
                            for s in range(QSUBS):
                                nc.tensor.matmul(
                                    otiles[s][:, h * E:(h + 1) * E],
                                    pm[:, u, s * P:(s + 1) * P],
                                    vp_sb[kti][:, h, :],
                                    start=(kti == 0), stop=(kti == NKT - 1))
                    pending_pv.append(pv_fn)
                    flush_pv(keep=PV_LAG)
                if h == 0 and pending_epi is not None:
                    pending_epi[1]()
                    pending_epi = None
                if h == 1 and qc + 1 < NQC:
                    # prefetch next chunk's mask AFTER the epilogue
                    # transposes so they don't queue behind 2MB of mask
                    # traffic on the sync engine
                    mk_next = dma_mask(qc + 1)
            flush_mul()
            flush_pv()
            pending_epi = make_epilogue(qc, q0, otiles)
        pending_epi[2]()

    nc.compile()
    return nc


_CACHE = {}


def _get_nc():
    if "nc" not in _CACHE:
        _CACHE["nc"] = build_kernel()
    return _CACHE["nc"]


def kernel(x, mask, Wq, Wkv, Wp, bp):
    x = np.asarray(x, np.float32)
    mask = np.asarray(mask, np.float32)
    Wq = np.asarray(Wq, np.float32)
    Wkv = np.asarray(Wkv, np.float32)
    Wp = np.asarray(Wp, np.float32)
    bp = np.asarray(bp, np.float32)

    nc = _get_nc()
    in_maps = []
    for core in range(NCORES):
        b, hg = divmod(core, HGROUPS)
        rows = slice(hg * CH, (hg + 1) * CH)
        in_maps.append({
            "xT": np.ascontiguousarray(x[b].T.astype(ml_dtypes.bfloat16)),
            "wqt": np.ascontiguousarray(
                ((Wq[rows, :] * SCALE).T).astype(ml_dtypes.bfloat16)),
            "wkt": np.ascontiguousarray(Wkv[rows, :].T.astype(ml_dtypes.bfloat16)),
            "wvt": np.ascontiguousarray(Wkv.T[:, C + hg * CH:C + (hg + 1) * CH].astype(ml_dtypes.bfloat16)),
            "wpt": np.ascontiguousarray(Wp[:, rows].T.astype(ml_dtypes.bfloat16)),
            "negmt": np.ascontiguousarray(
                (1.0 - mask[b].T).astype(ml_dtypes.bfloat16)),
        })

    trace = os.environ.get("KERNEL_TRACE", "0") == "1"
    if os.environ.get("KERNEL_WARMUP", "1") == "1":
        run_bass_kernel_spmd(nc, in_maps, core_ids=list(range(NCORES)),
                             trace=False)
    res = run_bass_kernel_spmd(nc, in_maps, core_ids=list(range(NCORES)),
                               trace=trace)
    kernel.last_results = res

    outs = [res.results[i]["out"] for i in range(NCORES)]
    full = np.empty((B, N, C), np.float32)
    for b in range(B):
        full[b] = outs[2 * b] + outs[2 * b + 1] + bp[None, :]
    return full
